# revision 30
# baseline (speedup 1.0000x reference)
"""Fused LN + RoPE multi-head attention for Trainium2, SPMD over 8 NeuronCores.

Problem: nn_MultiHeadAttention (B=4, S=2048, D=1024, H=16, Dh=64), fp32 I/O.

Sharding (per spec hint): data-parallel over batch x tensor-parallel over heads.
Core c handles batch b = c//2 and head-group g = c%2 (8 of 16 heads):
  - w_qkv column-sharded (this group's Q/K/V columns), ln_gamma folded in
  - w_o row-sharded
  - on-device ReduceScatter(add) over pairs {2b, 2b+1} after the output
    projection; host concatenates the scattered halves (pure gather).

v2.1 pipeline (single Tile context), engineered so ScalarE (exp) saturates:
  A) LayerNorm (bn_stats) token-major; PE-transpose -> xnT [D, S] (fp16),
     interleaved with the first head-pair's K^T/Q^T projections and V so the
     exp stream starts as early as possible.
  B) Per head-pair cb: K^T/Q^T via PE; RoPE rotate_half applied with a small
     block-diagonal permutation matmul instead of a second full projection;
     combine q*cos + rot*sin on DVE.  Head-pair cb+1 is built in 4-MM slices
     inside head-pair cb's jb loop, sized to fit the PE slack under the exp
     stream without stalling it (2 spare PSUM banks).
  C) Attention per (pair, q-quarter): scores^T [j, q] for both heads via
     row-tiled concurrent MMs (K=64 at partition offsets 0/64); ONE exp per
     (pair, qq, jb) over [128, 1024] covering both heads; AV with a
     ones-column appended to V so softmax denominators fall out (row 64);
     normalization via reciprocal_approx_fast + DMA-broadcast.
  D) Output projection sliced by q-quarter and interleaved into the last
     head-pair's loop; fp16 ReduceScatter per q-quarter overlaps the
     remaining attention + projection; DMA out y^T half [512, 2048].
"""

import numpy as np

import concourse.bacc as bacc
import concourse.mybir as mybir
import concourse.tile as tile
from concourse.bass_utils import run_bass_kernel_spmd
from concourse.masks import make_identity

F32 = mybir.dt.float32
F16 = mybir.dt.float16

B, S, D = 4, 2048, 1024
H, DH = 16, 64          # global heads
HL = 8                  # heads per core
N_CORES = 8
LN_EPS = 1e-5
SB = S // 128           # 16 s-blocks
DC = D // 128           # 8 d-chunks
CB = 4                  # head-pairs per core (2 heads / 128 partitions each)

_CACHE = {}


def _build():
    if "nc" in _CACHE:
        return _CACHE["nc"]
    nc = bacc.Bacc("TRN2", target_bir_lowering=False, debug=False,
                   num_devices=N_CORES)
    AF = mybir.ActivationFunctionType
    OP = mybir.AluOpType

    x_d = nc.dram_tensor("x", [S, D], F32, kind="ExternalInput").ap()
    wqkv_d = nc.dram_tensor("wqkv", [D, 3 * 512], F16, kind="ExternalInput").ap()
    prot_d = nc.dram_tensor("prot", [128, 128], F16, kind="ExternalInput").ap()
    wo_d = nc.dram_tensor("wo", [512, D], F16, kind="ExternalInput").ap()
    cos_d = nc.dram_tensor("cos2t", [128, S], F32, kind="ExternalInput").ap()
    sin_d = nc.dram_tensor("sin2t", [128, S], F32, kind="ExternalInput").ap()
    y_d = nc.dram_tensor("y", [512, S], F16, kind="ExternalOutput").ap()

    with tile.TileContext(nc) as tc:
        with (
            tc.tile_pool(name="singles", bufs=1) as singles,
            tc.tile_pool(name="persist", bufs=1) as persist,
            tc.tile_pool(name="dram", bufs=1, space="DRAM") as dram,
            tc.tile_pool(name="lnp", bufs=3) as lnp,
            tc.tile_pool(name="wpool", bufs=1) as wpool,
            tc.tile_pool(name="wop", bufs=1) as wop,
            tc.tile_pool(name="psB", bufs=1, space="PSUM") as psB,
            tc.tile_pool(name="ropep", bufs=2) as ropep,
        ):
            id_sb = singles.tile([128, 128], F16)
            eps_t = singles.tile([128, 1], F32)
            cos_sb = singles.tile([128, S], F32)
            sin_sb = singles.tile([128, S], F32)
            prot_sb = singles.tile([128, 128], F16)

            # persistent activations
            xnT = [persist.tile([128, S], F16, tag=f"xnT{i}", name=f"xnT{i}")
                   for i in range(DC)]
            QT = [persist.tile([128, S], F16, tag=f"QT{i}", name=f"QT{i}")
                  for i in range(CB)]
            KT = [persist.tile([128, S], F16, tag=f"KT{i}", name=f"KT{i}")
                  for i in range(CB)]
            V_ext = [persist.tile([128, HL, DH + 1], F16, tag=f"V{i}", name=f"V{i}")
                     for i in range(SB)]
            outn = [persist.tile([128, S], F16, tag=f"on{i}", name=f"on{i}")
                    for i in range(CB)]
            rec_dram = dram.tile([32, 512], F32)
            wqkv_sb = [wpool.tile([128, 3 * 512], F16, tag=f"wq{i}", name=f"wq{i}")
                       for i in range(DC)]
            wo_sb = [wop.tile([128, D], F16, tag=f"wo{i}", name=f"wo{i}")
                     for i in range(4)]

            # x-blocks 0-2 queued before the bulky weight DMAs
            x_tiles = []
            for sb in range(3):
                x_t = lnp.tile([128, D], F32, tag="x")
                nc.sync.dma_start(x_t, x_d[sb * 128:(sb + 1) * 128, :])
                x_tiles.append(x_t)
            make_identity(nc, id_sb)
            nc.vector.memset(eps_t, LN_EPS)
            nc.sync.dma_start(cos_sb, cos_d)
            nc.sync.dma_start(sin_sb, sin_d)
            nc.sync.dma_start(prot_sb, prot_d)
            for dc in range(DC):
                nc.sync.dma_start(wqkv_sb[dc], wqkv_d[dc * 128:(dc + 1) * 128, :])

            def build_units(cb):
                """K^T/Q^T (+RoPE) for head-pair cb, yielded in ~4-MM units.
                Chunk-major (K then Q per 512-col chunk) so phase A can
                consume chunk g right after LayerNorm of s-blocks 4g..4g+3."""
                for n in range(4):
                    nsl = slice(n * 512, (n + 1) * 512)
                    for wcol0, dstT in ((512, KT), (0, QT)):
                        wsl = slice(wcol0 + cb * 128, wcol0 + (cb + 1) * 128)
                        qk = psB.tile([128, 512], F32, tag="qk")
                        for dc in range(DC):
                            nc.tensor.matmul(qk, wqkv_sb[dc][:, wsl],
                                             xnT[dc][:, nsl],
                                             start=(dc == 0),
                                             stop=(dc == DC - 1),
                                             skip_group_check=True)
                            if dc % 2 == 1 and dc < DC - 1:
                                yield
                        raw = ropep.tile([128, 512], F16, tag="raw")
                        nc.vector.tensor_copy(raw, qk)
                        ca = ropep.tile([128, 512], F16, tag="ca")
                        nc.vector.tensor_mul(ca, qk, cos_sb[:, nsl])
                        rot = psB.tile([128, 512], F32, tag="rot")
                        nc.tensor.matmul(rot, prot_sb, raw, start=True, stop=True,
                                         skip_group_check=True)
                        cbt = ropep.tile([128, 512], F16, tag="cb")
                        nc.vector.tensor_mul(cbt, rot, sin_sb[:, nsl])
                        nc.vector.tensor_add(dstT[cb][:, nsl], ca, cbt)
                        yield

            def emit_v(sb, drain_scalar):
                vp = psV.tile([128, 512], F32, tag="v")
                for dc in range(DC):
                    nc.tensor.matmul(vp, xnT[dc][:, sb * 128:(sb + 1) * 128],
                                     wqkv_sb[dc][:, 1024:1536],
                                     start=(dc == 0), stop=(dc == DC - 1),
                                     skip_group_check=True)
                nc.vector.memset(V_ext[sb][:, :, DH:DH + 1], 1.0)
                dst = V_ext[sb][:, :, 0:DH]
                src = vp.rearrange("p (h d) -> p h d", h=HL)
                if drain_scalar:
                    nc.scalar.activation(out=dst, in_=src, func=AF.Copy)
                else:
                    nc.vector.tensor_copy(dst, src)

            # ---------- Phase A: LN + transpose, interleaved with pair-0 ----
            b0 = build_units(0)
            psV_cm = tc.tile_pool(name="psV", bufs=2, space="PSUM")
            psV = psV_cm.__enter__()
            with (
                tc.tile_pool(name="stats", bufs=4) as stats,
                tc.tile_pool(name="psA", bufs=4, space="PSUM") as psA,
            ):
                for sb in range(SB):
                    if sb < 3:
                        x_t = x_tiles[sb]
                    else:
                        x_t = lnp.tile([128, D], F32, tag="x")
                        nc.sync.dma_start(x_t, x_d[sb * 128:(sb + 1) * 128, :])
                    st = stats.tile([128, 2, nc.vector.BN_STATS_DIM], F32, tag="st")
                    nc.vector.bn_stats(st[:, 0, :], x_t[:, 0:512])
                    nc.vector.bn_stats(st[:, 1, :], x_t[:, 512:1024])
                    mv = stats.tile([128, nc.vector.BN_AGGR_DIM], F32, tag="mv")
                    nc.vector.bn_aggr(mv, st)
                    sd = stats.tile([128, 1], F32, tag="sd")
                    nc.scalar.activation(out=sd, in_=mv[:, 1:2], func=AF.Sqrt,
                                         bias=eps_t, scale=1.0)
                    rstd = stats.tile([128, 1], F32, tag="rstd")
                    nc.vector.reciprocal(rstd, sd)
                    xn_t = lnp.tile([128, D], F16, tag="xn")
                    nc.vector.tensor_scalar(out=xn_t, in0=x_t,
                                            scalar1=mv[:, 0:1], scalar2=rstd,
                                            op0=OP.subtract, op1=OP.mult)
                    for dc in range(DC):
                        tr = psA.tile([128, 128], F16, tag="tr")
                        nc.tensor.transpose(tr, xn_t[:, dc * 128:(dc + 1) * 128],
                                            id_sb)
                        dst = xnT[dc][:, sb * 128:(sb + 1) * 128]
                        if dc % 2 == 0:
                            nc.vector.tensor_copy(dst, tr)
                        else:
                            nc.scalar.activation(out=dst, in_=tr, func=AF.Copy)
                    if sb % 4 == 3:
                        # K+Q chunk for s-cols of this group (4 units)
                        for _ in range(4):
                            next(b0, None)
                        emit_v(sb - 3, drain_scalar=True)
                        emit_v(sb - 2, drain_scalar=False)
            for _ in b0:
                pass
            for sb in range(SB):
                if sb % 4 >= 2:
                    emit_v(sb, drain_scalar=(sb % 2 == 1))
            psV_cm.__exit__(None, None, None)
            for kc in range(4):
                nc.sync.dma_start(wo_sb[kc], wo_d[kc * 128:(kc + 1) * 128, :])

            # ---------- Phase C + build-ahead + phase D interleaved ----------
            rs_in = [dram.tile([D, 512], F16, tag=f"rsi{i}", name=f"rsi{i}")
                     for i in range(4)]
            rs_out = [dram.tile([512, 512], F16, tag=f"rso{i}", name=f"rso{i}")
                      for i in range(4)]
            with (
                tc.tile_pool(name="expp", bufs=2) as expp,
                tc.tile_pool(name="avp", bufs=8) as avp,
                tc.tile_pool(name="nrm", bufs=2) as nrm,
                tc.tile_pool(name="yp", bufs=2) as ypool,
                tc.tile_pool(name="pssc", bufs=2, space="PSUM") as pssc,
                tc.tile_pool(name="psav", bufs=2, space="PSUM") as psav,
            ):
                def d_chunk(qq, ob, drain_scalar):
                    """Output projection for q-quarter qq, row block ob."""
                    qsl = slice(qq * 512, (qq + 1) * 512)
                    yp = psB.tile([128, 512], F32, tag=("qk", "rot")[ob % 2])
                    for kc in range(4):
                        nc.tensor.matmul(yp, wo_sb[kc][:, ob * 128:(ob + 1) * 128],
                                         outn[kc][:, qsl],
                                         start=(kc == 0), stop=(kc == 3),
                                         skip_group_check=True)
                    ysb = ypool.tile([128, 512], F16, tag="ysb")
                    if drain_scalar:
                        nc.scalar.activation(out=ysb, in_=yp, func=AF.Copy)
                    else:
                        nc.vector.tensor_copy(ysb, yp)
                    nc.sync.dma_start(rs_in[qq][ob * 128:(ob + 1) * 128, :], ysb)

                def rs_chunk(qq):
                    nc.gpsimd.collective_compute(
                        "ReduceScatter",
                        mybir.AluOpType.add,
                        replica_groups=[[0, 1], [2, 3], [4, 5], [6, 7]],
                        ins=[rs_in[qq][:].opt()],
                        outs=[rs_out[qq][:].opt()],
                    )
                    nc.sync.dma_start(y_d[:, qq * 512:(qq + 1) * 512], rs_out[qq])

                builder = None
                for cb in range(CB):
                    avs_saved = {}
                    rstage = nrm.tile([8, 512], F32, tag="rst8", bufs=1)
                    if cb < CB - 1:
                        builder = build_units(cb + 1)
                    for qq in range(4):
                        qsl = slice(qq * 512, (qq + 1) * 512)
                        av0 = psav.tile([65, 512], F32, tag="av")
                        av1 = psav.tile([65, 512], F32, tag="av")
                        for jb in range(SB):
                            jsl = slice(jb * 128, (jb + 1) * 128)
                            sc = pssc.tile([128, 1024], F32, tag="sc")
                            nc.tensor.matmul(sc[:, 0:512],
                                             KT[cb][0:64, jsl], QT[cb][0:64, qsl],
                                             start=True, stop=True,
                                             skip_group_check=True)
                            nc.tensor.matmul(sc[:, 512:1024],
                                             KT[cb][64:128, jsl],
                                             QT[cb][64:128, qsl],
                                             start=True, stop=True,
                                             skip_group_check=True)
                            ex = expp.tile([128, 1024], F16, tag="ex")
                            nc.scalar.activation(out=ex, in_=sc, func=AF.Exp,
                                                 scale=0.125)
                            nc.tensor.matmul(av0, V_ext[jb][:, 2 * cb, :],
                                             ex[:, 0:512],
                                             start=(jb == 0), stop=(jb == SB - 1),
                                             skip_group_check=True)
                            nc.tensor.matmul(av1, V_ext[jb][:, 2 * cb + 1, :],
                                             ex[:, 512:1024],
                                             start=(jb == 0), stop=(jb == SB - 1),
                                             skip_group_check=True)
                            # spread next pair's projections in the PE slack
                            if builder is not None and (qq * SB + jb) % 8 < 5:
                                next(builder, None)
                                if qq == 3 and jb == 12:
                                    for _ in builder:
                                        pass
                                    builder = None
                            # last pair: interleave the output projection
                            # (from jb 6 so the normalize chain clears first)
                            if cb == CB - 1 and qq > 0 and 6 <= jb < 14:
                                d_chunk(qq - 1, jb - 6, drain_scalar=False)
                        # drain accumulators; stage rowsums for batched recip
                        if cb == CB - 1:
                            rst2 = nrm.tile([2, 512], F32, tag="rst2", bufs=1)
                        for h2, av in ((0, av0), (1, av1)):
                            avs = avp.tile([65, 512], F32, tag="avs")
                            nc.vector.tensor_copy(avs, av)
                            avs_saved[(qq, h2)] = avs
                            dst = (rst2[h2:h2 + 1, :] if cb == CB - 1 else
                                   rstage[qq * 2 + h2:qq * 2 + h2 + 1, :])
                            nc.sync.dma_start(dst, avs[64:65, :])

                        def bcast_mul(nqq):
                            r0 = cb * 8 + nqq * 2
                            for h2 in range(2):
                                bc = nrm.tile([64, 512], F32, tag="bc")
                                nc.sync.dma_start(
                                    bc, rec_dram[r0 + h2:r0 + h2 + 1, :]
                                    .to_broadcast((64, 512)))
                                dsl = (slice(nqq * 512, (nqq + 1) * 512))
                                if h2 == 0:
                                    nc.vector.tensor_mul(
                                        outn[cb][0:64, dsl],
                                        avs_saved[(nqq, 0)][0:64, :], bc)
                                else:
                                    on = nrm.tile([64, 512], F16, tag="on")
                                    nc.vector.tensor_mul(
                                        on, avs_saved[(nqq, 1)][0:64, :], bc)
                                    nc.sync.dma_start(outn[cb][64:128, dsl], on)

                        if cb == CB - 1:
                            # last pair: normalize per qq, D consumes it next
                            rr = nrm.tile([2, 512], F32, tag="rr", bufs=1)
                            nc.vector.reciprocal(rr, rst2)
                            nc.sync.dma_start(
                                rec_dram[cb * 8 + qq * 2:cb * 8 + qq * 2 + 2, :],
                                rr)
                            bcast_mul(qq)
                            if qq > 0:
                                rs_chunk(qq - 1)
                        elif qq == 3:
                            # one batched reciprocal for the whole pair
                            rr8 = nrm.tile([8, 512], F32, tag="rr8", bufs=1)
                            nc.vector.reciprocal(rr8, rstage)
                            nc.sync.dma_start(
                                rec_dram[cb * 8:cb * 8 + 8, :], rr8)
                            for nqq in range(4):
                                bcast_mul(nqq)
                # tail: projection for the final q-quarter + its collective
                for ob in range(8):
                    d_chunk(3, ob, drain_scalar=(ob % 2 == 1))
                rs_chunk(3)

    nc.compile()
    _CACHE["nc"] = nc
    return nc


def _make_prot():
    """lhsT for the rotate_half matmul: out = prot.T @ qT applies, per 64-row
    head block, out[d] = -q[d+32] (d<32) / q[d-32] (d>=32)."""
    P = np.zeros((128, 128), np.float32)
    for i in range(128):
        if i % 64 < 32:
            P[i, i + 32] = 1.0
        else:
            P[i, i - 32] = -1.0
    return P.astype(np.float16)


def _prep_inputs(inputs, cos, sin, ln_gamma, w_qkv, w_o):
    x = np.asarray(inputs, np.float32)
    cos = np.asarray(cos, np.float32)
    sin = np.asarray(sin, np.float32)
    wg = np.asarray(w_qkv, np.float32) * np.asarray(ln_gamma, np.float32)[:, None]
    w_o = np.asarray(w_o, np.float32)
    wq, wk, wv = wg[:, 0:D], wg[:, D:2 * D], wg[:, 2 * D:3 * D]
    ct = np.ascontiguousarray(cos.T)          # [64, S]
    st = np.ascontiguousarray(sin.T)
    cos2t = np.concatenate([ct, ct], 0)       # [128, S]
    sin2t = np.concatenate([st, st], 0)
    prot = _make_prot()
    in_maps = []
    for c in range(N_CORES):
        b, g = c // 2, c % 2
        gs = slice(g * 512, (g + 1) * 512)
        in_maps.append({
            "x": np.ascontiguousarray(x[b]),
            "wqkv": np.ascontiguousarray(
                np.concatenate([wq[:, gs], wk[:, gs], wv[:, gs]], 1)
            ).astype(np.float16),
            "prot": prot,
            "wo": np.ascontiguousarray(w_o[gs, :]).astype(np.float16),
            "cos2t": cos2t,
            "sin2t": sin2t,
        })
    return in_maps


def _ensure_ntff_hook():
    """The agent image's antenv lacks axon_hooks; shim it and register the
    ctypes NTFF hook against the injected libaxon_pjrt.so so trace=True works."""
    import sys
    import types
    if "antenv.axon_hooks" in sys.modules:
        return
    mod = types.ModuleType("antenv.axon_hooks")
    state = {"hook": None}
    mod.set_axon_ntff_profile_hook = lambda h: state.__setitem__("hook", h)
    mod.get_axon_ntff_profile_hook = lambda: state["hook"]
    sys.modules["antenv.axon_hooks"] = mod
    try:
        import antenv
        antenv.axon_hooks = mod
    except ImportError:
        pass
    try:
        from trn_agent_boot.trn_boot import _ntff_profile_via_ctypes
        mod.set_axon_ntff_profile_hook(
            _ntff_profile_via_ctypes("/opt/axon/libaxon_pjrt.so"))
    except Exception:
        pass


def _run(in_maps, trace=False):
    nc = _build()
    if trace:
        _ensure_ntff_hook()
    return run_bass_kernel_spmd(nc, in_maps, core_ids=list(range(N_CORES)),
                                trace=trace)


def _assemble(results):
    out = np.empty((B, S, D), np.float32)
    for b in range(B):
        yT = np.concatenate([
            np.asarray(results[2 * b]["y"], np.float32),
            np.asarray(results[2 * b + 1]["y"], np.float32)], 0)
        out[b] = yT.T
    return out


def kernel(inputs, mask, cos, sin, ln_gamma, w_qkv, w_o):
    in_maps = _prep_inputs(inputs, cos, sin, ln_gamma, w_qkv, w_o)
    res = _run(in_maps, trace=False)
    return _assemble(res.results)


def kernel_traced(inputs, mask, cos, sin, ln_gamma, w_qkv, w_o):
    """Like kernel() but also returns the BassKernelResults (exec_time_ns)."""
    in_maps = _prep_inputs(inputs, cos, sin, ln_gamma, w_qkv, w_o)
    res = _run(in_maps, trace=True)
    return _assemble(res.results), res
